# revision 28
# baseline (speedup 1.0000x reference)
"""Low-rank RNN Bass kernel for Trainium2 (8 NeuronCores, data-parallel over batch).

Model (per reference):
  inp_t = u_t @ Win_w.T + Win_b
  r     = tanh(x)
  rec   = (r @ N) @ M.T / H + inp_t
  x'    = 0.8*x + [0.2*inp_t + 0.05*n_t] + (r @ N) @ (0.2/H * M).T
  out   = tanh(traj) @ Wout_w.T + Wout_b

Device layout per core (batch slice of 8):
  state x transposed: [128 partitions = h%128, free = 8 h-chunks x 8 batch]
  per-step chain (4 engine hops):
    ACT  tanh(x) -> rbuf slot
    PE   z_rep[32,8]  = sum_c N1rep_c^T @ r_c   (8 accum matmuls, replicated x8)
    DVE  Z[32,64]     = bcast(z_rep) * Qmask    (block-diagonal z)
    PE   x'[128,64]   = Mstack^T @ Z + I^T @ (0.8x + c_t)   (2 matmuls, one PSUM group)
  the affine term t1 = 0.8*x + c_t is built by DVE off the critical path;
  drive c_t = 0.2*(Win u + b) + 0.05*n computed per 32-step window on PE+DVE,
  overlapped with the scan; output projection matmuls interleaved into PE
  idle slots of the scan.
"""

import numpy as np

B, T, I, H, O, R = 64, 512, 16, 1024, 8, 4
NCORES = 8
BL = B // NCORES          # 8 batch per core
HC = H // 128             # 8 h-chunks
CB = HC * BL              # 64 = free width of one state tile
WIN = 32                  # steps per drive window
NWIN = T // WIN
OG = 64                   # steps per output-projection PSUM group
NOISE_STD = 0.05
TAU = 0.2

_cache = {}


MM2_MODE = "v1bf16"  # v1bf16 (8 bf16 chunk-mms + DVE add, HW-verified 1.144ms)
                     # | hilo (bf16 mm2 + hi/lo identity-mm fold) | f32
OGRAN = 4            # steps per output-projection matmul batch
OUTPROJ_MID = False  # emit outproj matmuls between mm1 and mm2 (fill Zbuild wait)


def _build(timing_reps=1, mm2_mode=None, outproj_mid=None):
    """timing_reps > 1 wraps the whole computation in a HW For_i loop so
    device time can be measured from wall-clock deltas (bench only)."""
    from contextlib import nullcontext

    import concourse.bacc as bacc
    import concourse.mybir as mybir
    import concourse.tile as tile
    import concourse.bass as bass

    mm2_mode = MM2_MODE if mm2_mode is None else mm2_mode
    outproj_mid = OUTPROJ_MID if outproj_mid is None else outproj_mid

    FP = mybir.dt.float32
    Tanh = mybir.ActivationFunctionType.Tanh
    mult = mybir.AluOpType.mult
    add = mybir.AluOpType.add

    nc = bacc.Bacc("TRN2", target_bir_lowering=False, debug=False)

    BF = mybir.dt.bfloat16

    uT_d = nc.dram_tensor("uT", [I + 1, T * BL], FP, kind="ExternalInput")
    x0T_d = nc.dram_tensor("x0T", [128, CB], FP, kind="ExternalInput")
    noise_d = nc.dram_tensor("noiseT", [128, T * CB], FP, kind="ExternalInput")
    n1_d = nc.dram_tensor("N1", [128, HC * 8 * R], FP, kind="ExternalInput")
    n1p_d = nc.dram_tensor("N1p", [128, HC * R], FP, kind="ExternalInput")
    m2b_d = nc.dram_tensor("M2b", [R, H], BF, kind="ExternalInput")
    ms_d = nc.dram_tensor("Mstack", [8 * R, 128], FP, kind="ExternalInput")
    q_d = nc.dram_tensor("Qmask", [8 * R, CB], FP, kind="ExternalInput")
    id_d = nc.dram_tensor("Ident", [128, 128], FP, kind="ExternalInput")
    winT_d = nc.dram_tensor("WinT", [I + 1, H], FP, kind="ExternalInput")
    woutT_d = nc.dram_tensor("WoutT", [128, HC * O], FP, kind="ExternalInput")
    woutb_d = nc.dram_tensor("Woutb", [O, 1], FP, kind="ExternalInput")
    out_d = nc.dram_tensor("outT", [O, T * BL], FP, kind="ExternalOutput")

    with tile.TileContext(nc) as tc:
        with (
            tc.tile_pool(name="const", bufs=1) as constp,
            tc.tile_pool(name="rbufp", bufs=1) as rbufp,
            tc.tile_pool(name="winp", bufs=2) as winp,
            tc.tile_pool(name="smallp", bufs=3) as smallp,
            tc.tile_pool(name="outp", bufs=2) as outp,
            tc.tile_pool(name="psx", bufs=2, space="PSUM") as psx,
            tc.tile_pool(name="pso", bufs=2, space="PSUM") as pso,
            tc.tile_pool(name="psz", bufs=2, space="PSUM") as psz,
            tc.tile_pool(name="psw", bufs=2, space="PSUM") as psw,
        ):
            uT = constp.tile([I + 1, T * BL], FP)
            nc.sync.dma_start(uT[:], uT_d[:])
            x0T = constp.tile([128, CB], FP)
            nc.sync.dma_start(x0T[:], x0T_d[:])
            N1 = constp.tile([128, HC * 8 * R], FP)
            nc.sync.dma_start(N1[:], n1_d[:])
            N1p = constp.tile([128, HC * R], FP)
            nc.sync.dma_start(N1p[:], n1p_d[:])
            M2b = constp.tile([R, H], BF)
            nc.sync.dma_start(M2b[:], m2b_d[:])
            Mstack = constp.tile([8 * R, 128], FP)
            nc.sync.dma_start(Mstack[:], ms_d[:])
            Qmask = constp.tile([8 * R, CB], FP)
            nc.sync.dma_start(Qmask[:], q_d[:])
            Ident = constp.tile([128, 128], FP)
            nc.sync.dma_start(Ident[:], id_d[:])
            IdentB = constp.tile([128, 128], BF)
            nc.vector.tensor_copy(IdentB[:], Ident[:])
            WinT = constp.tile([I + 1, H], FP)
            nc.sync.dma_start(WinT[:], winT_d[:])
            WoutT = constp.tile([128, HC * O], FP)
            nc.sync.dma_start(WoutT[:], woutT_d[:])
            Woutb = constp.tile([O, 1], FP)
            nc.sync.dma_start(Woutb[:], woutb_d[:])

            # tanh(x_t) for t = 0..T, slot t at [:, t*CB:(t+1)*CB]
            rbuf = rbufp.tile([128, (T + 1) * CB], FP)
            r4 = rbuf[:].rearrange("p (t c b) -> p t c b", c=HC, b=BL)

            q3 = Qmask[:].rearrange("p (c b) -> p c b", b=BL)

            cwins = {}

            def prep_dma(w):
                nw = winp.tile([128, WIN * CB], FP, tag="nw")
                nc.sync.dma_start(
                    nw[:], noise_d[:, w * WIN * CB : (w + 1) * WIN * CB]
                )
                cw = winp.tile([128, WIN * CB], FP, tag="cw")
                cwins[w] = (cw, nw)

            def prep_chunk(w, c, pspool):
                cw, nw = cwins[w]
                ps = pspool.tile([128, WIN * BL], FP, tag="psw")
                nc.tensor.matmul(
                    ps[:],
                    WinT[:, c * 128 : (c + 1) * 128],
                    uT[:, w * WIN * BL : (w + 1) * WIN * BL],
                    start=True,
                    stop=True,
                )
                csl = cw[:].rearrange("p (t c b) -> p t c b", c=HC, b=BL)[:, :, c, :]
                nsl = nw[:].rearrange("p (t c b) -> p t c b", c=HC, b=BL)[:, :, c, :]
                psl = ps[:].rearrange("p (t b) -> p t b", b=BL)
                nc.vector.scalar_tensor_tensor(
                    csl, nsl, NOISE_STD, psl, op0=mult, op1=add
                )

            # output projection group state
            po_tiles = {}

            og = min(OG, T)
            ogr = min(OGRAN, og)

            def outproj_batch(t0):
                """Project slots [t0, t0+ogr) into the PSUM group (8 matmuls)."""
                g, pos = (t0 - 1) // og, (t0 - 1) % og
                if pos == 0:
                    po_tiles[g] = pso.tile([O, og * BL], FP, tag="po", name="po")
                po = po_tiles[g]
                for c in range(HC):
                    nc.tensor.matmul(
                        po[:, pos * BL : (pos + ogr) * BL],
                        WoutT[:, c * O : (c + 1) * O],
                        r4[:, t0 : t0 + ogr, c, :],
                        start=(c == 0),
                        stop=(c == HC - 1),
                    )
                if pos + ogr == og:
                    ob = outp.tile([O, og * BL], FP, tag="ob")
                    nc.vector.tensor_scalar_add(ob[:], po[:], Woutb[:, 0:1])
                    nc.sync.dma_start(
                        out_d[:, g * og * BL : (g + 1) * og * BL], ob[:]
                    )

            loop_cm = (
                tc.For_i(0, timing_reps, 1) if timing_reps > 1 else nullcontext()
            )
            with loop_cm:
                prep_dma(0)
                for c in range(HC):
                    prep_chunk(0, c, psw)

                x_prev = x0T
                op_next = 1  # next unprojected rbuf slot
                for w in range(NWIN):
                    cw, _ = cwins[w]
                    if w + 1 < NWIN:
                        prep_dma(w + 1)
                    for ti in range(WIN):
                        t = w * WIN + ti + 1
                        # r_{t-1} = tanh(x_{t-1}) -> rbuf slot t-1
                        rslot = rbuf[:, (t - 1) * CB : t * CB]
                        nc.scalar.activation(rslot, x_prev[:], Tanh)
                        # t1 = 0.8*x_{t-1} + c_t   (off critical path)
                        t1 = smallp.tile([128, CB], FP, tag="t1")
                        ct = cw[:, ti * CB : (ti + 1) * CB]
                        nc.vector.scalar_tensor_tensor(
                            t1[:], x_prev[:], 1.0 - TAU, ct, op0=mult, op1=add
                        )
                        xn = psx.tile([128, CB], FP, tag="xn")
                        hilo = mm2_mode == "hilo"
                        if mm2_mode in ("v1bf16", "hilo"):
                            # z[4,8] = N^T r (8 accum mms), ACT-copy to bf16,
                            # then 8 bf16 expansion mms (FWL fast loads)
                            z = psz.tile([R, BL], FP, tag="z")
                            for c in range(HC):
                                nc.tensor.matmul(
                                    z[:],
                                    N1p[:, c * R : (c + 1) * R],
                                    rslot[:, c * BL : (c + 1) * BL],
                                    start=(c == 0),
                                    stop=(c == HC - 1),
                                )
                            zs = smallp.tile([R, BL], BF, tag="zs")
                            nc.scalar.copy(zs[:], z[:])
                            if hilo:
                                # fold t1 on PE via hi/lo bf16 split (exact to
                                # ~2^-18); hi-matmul opens the PSUM group early,
                                # off the critical path
                                hi = smallp.tile([128, CB], BF, tag="hi")
                                nc.scalar.copy(hi[:], t1[:])
                                lo = smallp.tile([128, CB], BF, tag="lo")
                                nc.vector.tensor_sub(lo[:], t1[:], hi[:])
                                nc.tensor.matmul(
                                    xn[:], IdentB[:], hi[:], start=True, stop=False
                                )
                            for c in range(HC):
                                nc.tensor.matmul(
                                    xn[:, c * BL : (c + 1) * BL],
                                    M2b[:, c * 128 : (c + 1) * 128],
                                    zs[:],
                                    start=not hilo,
                                    stop=False if hilo else True,
                                )
                            if hilo:
                                nc.tensor.matmul(
                                    xn[:], IdentB[:], lo[:], start=False, stop=True
                                )
                            else:
                                nc.vector.tensor_add(xn[:], xn[:], t1[:])
                        else:  # "f32": block-diagonal Z + fp32 identity matmul
                            z = psz.tile([8 * R, BL], FP, tag="z")
                            for c in range(HC):
                                nc.tensor.matmul(
                                    z[:],
                                    N1[:, c * 8 * R : (c + 1) * 8 * R],
                                    rslot[:, c * BL : (c + 1) * BL],
                                    start=(c == 0),
                                    stop=(c == HC - 1),
                                )
                            Zt = smallp.tile([8 * R, CB], FP, tag="Zt")
                            zap = z[:]
                            zb = bass.AP(
                                zap.tensor, zap.offset, [zap.ap[0], [0, HC], zap.ap[1]]
                            )
                            nc.vector.tensor_tensor(
                                Zt[:].rearrange("p (c b) -> p c b", b=BL),
                                zb,
                                q3,
                                op=mult,
                            )
                            nc.tensor.matmul(
                                xn[:], Mstack[:], Zt[:], start=True, stop=False
                            )
                            nc.tensor.matmul(
                                xn[:], Ident[:], t1[:], start=False, stop=True
                            )
                        x_prev = xn
                        # interleave: project any complete batch of ready slots
                        while op_next + ogr - 1 <= t - 1:
                            outproj_batch(op_next)
                            op_next += ogr
                        # spread next window's drive prep over this window
                        if w + 1 < NWIN:
                            for c in range(ti * HC // WIN, (ti + 1) * HC // WIN):
                                prep_chunk(w + 1, c, psw)

                # final tanh: slot T = tanh(x_T)
                nc.scalar.activation(rbuf[:, T * CB : (T + 1) * CB], x_prev[:], Tanh)
                while op_next <= T:
                    outproj_batch(op_next)
                    op_next += ogr

    nc.compile()
    return nc


def _get_nc():
    if "nc" not in _cache:
        _cache["nc"] = _build()
    return _cache["nc"]


def _host_prep(u, x0, noise, M, N, Win_w, Win_b, Wout_w, Wout_b):
    """Per-core input maps (layout/transpose only + constant folds on params)."""
    import ml_dtypes

    f = np.float32
    # N1rep: [p, c*(8R) + (c2*R + r)] = N[c*128+p, r]  (replicated 8x along c2)
    n_chunks = N.reshape(HC, 128, R).transpose(1, 0, 2)  # [p, c, r]
    N1 = np.ascontiguousarray(
        np.tile(n_chunks[:, :, None, :], (1, 1, 8, 1)).reshape(128, HC * 8 * R),
        dtype=f,
    )
    N1p = np.ascontiguousarray(n_chunks.reshape(128, HC * R), dtype=f)
    M2b = np.ascontiguousarray((TAU / H) * M.T).astype(ml_dtypes.bfloat16)  # [R, H]
    # Mstack: [(c, r), m] = (TAU/H) * M[c*128+m, r]
    Mpp = (TAU / H) * M.astype(np.float64)
    Mstack = np.ascontiguousarray(
        Mpp.reshape(HC, 128, R).transpose(0, 2, 1).reshape(HC * R, 128), dtype=f
    )
    # Qmask: [(c', r), (c, b)] = 1 if c == c'
    Qmask = np.zeros((HC * R, HC * BL), dtype=f)
    for c in range(HC):
        Qmask[c * R : (c + 1) * R, c * BL : (c + 1) * BL] = 1.0
    Ident = np.eye(128, dtype=f)
    WinT = np.concatenate(
        [TAU * Win_w.T.astype(f), (TAU * Win_b).astype(f)[None, :]], axis=0
    )  # [I+1, H]
    WoutT = np.ascontiguousarray(
        Wout_w.T.reshape(HC, 128, O).transpose(1, 0, 2).reshape(128, HC * O), dtype=f
    )
    Woutb = np.ascontiguousarray(Wout_b.astype(f)[:, None])

    in_maps = []
    for k in range(NCORES):
        bs = slice(k * BL, (k + 1) * BL)
        u_l = u[bs]  # (BL, T, I)
        uT = np.concatenate(
            [
                np.ascontiguousarray(u_l.transpose(2, 1, 0)).reshape(I, T * BL),
                np.ones((1, T * BL), dtype=f),
            ],
            axis=0,
        ).astype(f)
        x0T = np.ascontiguousarray(
            x0[bs].T.reshape(HC, 128, BL).transpose(1, 0, 2).reshape(128, CB), dtype=f
        )
        n_l = noise[:, bs, :]  # (T, BL, H)
        noiseT = np.ascontiguousarray(
            n_l.reshape(T, BL, HC, 128).transpose(3, 0, 2, 1).reshape(128, T * CB),
            dtype=f,
        )
        in_maps.append(
            {
                "uT": uT,
                "x0T": x0T,
                "noiseT": noiseT,
                "N1": N1,
                "N1p": N1p,
                "M2b": M2b,
                "Mstack": Mstack,
                "Qmask": Qmask,
                "Ident": Ident,
                "WinT": WinT,
                "WoutT": WoutT,
                "Woutb": Woutb,
            }
        )
    return in_maps


last_results = None


def kernel(u, x0, noise, M, N, Win_w, Win_b, Wout_w, Wout_b):
    from concourse.bass_utils import run_bass_kernel_spmd

    global last_results
    nc = _get_nc()
    in_maps = _host_prep(u, x0, noise, M, N, Win_w, Win_b, Wout_w, Wout_b)
    res = run_bass_kernel_spmd(nc, in_maps, core_ids=list(range(NCORES)))
    last_results = res
    out = np.empty((B, T, O), dtype=np.float32)
    for k in range(NCORES):
        outT = res.results[k]["outT"]  # [O, T*BL]
        out[k * BL : (k + 1) * BL] = outT.reshape(O, T, BL).transpose(2, 1, 0)
    return out
